# revision 26
# baseline (speedup 1.0000x reference)
"""MoE (N=16384, D=512, E=8, top_k=2) on 8 trn2 NeuronCores.

Strategy: group tokens globally by their unordered expert pair {e_a, e_b}
(28 groups for E=8), shard every group evenly across the 8 cores (96 slots
per core-group segment). Each core runs an identical (SPMD) program: 28
tiles of 96 tokens, each doing 8 accumulating float32r matmuls (2 experts
x 4 K-chunks, moving free dim 512) into two PSUM banks, then a
per-partition gate-weighted combine split across the Scalar and Vector
engines. All routing data-dependence lives in the host-side input
arrangement; the device program is fixed.
"""

import numpy as np

# ---------------------------------------------------------------------------
# The walrus build in this image accepts at most ONE sync-wait command per
# instruction, while Tile's semaphore assignment attaches several (DMA WAR +
# producer sems, and the kernel-tail drain waits on every live proc). Post-
# pass over the finished BIR: any instruction carrying more than one wait is
# preceded by same-engine nops that each take one wait. The engine executes
# its queue in order, so semantics are unchanged.
import bass_rust

_MAX_WAITS = 1


def _split_multi_waits(nc):
    for f in nc.m.functions:
        for blk in f.blocks:
            insts = blk.instructions
            k = 0
            while k < len(insts):
                inst = insts[k]
                si = getattr(inst, "sync_info", None)
                if si is not None and si.on_wait and len(si.on_wait) > _MAX_WAITS:
                    waits = list(si.on_wait)
                    keep = waits[-_MAX_WAITS:]
                    extra = waits[:-_MAX_WAITS]
                    inst.sync_info = bass_rust.SyncInfo(
                        on_wait=keep, on_update=list(si.on_update)
                    )
                    for j, i0 in enumerate(range(0, len(extra), _MAX_WAITS)):
                        nop = bass_rust.InstNoOp(
                            name=f"{inst.name}-wsplit{j}", ins=[], outs=[]
                        )
                        nop.engine = inst.engine
                        nop.sync_info = bass_rust.SyncInfo(
                            on_wait=extra[i0 : i0 + _MAX_WAITS], on_update=[]
                        )
                        insts.insert(k, nop)
                        k += 1
                k += 1
# ---------------------------------------------------------------------------

# Re-enable walrus's LDWEIGHTS dedup/pipelining pass: fp32r matmuls self-load
# their stationary operand, which would otherwise serialize with every
# matmul stream.
import concourse.bass_utils as _bu

if not getattr(_bu, "_ldw_opt_patched", False):
    _orig_run_command = _bu.run_command

    def _run_command_ldw(cmd, **kw):
        cmd = [
            "--enable-ldw-opt=true" if c == "--enable-ldw-opt=false" else c
            for c in cmd
        ]
        return _orig_run_command(cmd, **kw)

    _bu.run_command = _run_command_ldw
    _bu._ldw_opt_patched = True

import concourse.bass as bass
import concourse.mybir as mybir
from concourse.tile import TileContext
from concourse.bass_utils import run_bass_kernel_spmd

N, D, E, TOPK = 16384, 512, 8, 2
NCORES = 8
# triangle order: pair {a,b} (a<b) sorted by max expert, so expert m is
# first needed late and its weights can stream in during compute
PAIRS = [(a, b) for b in range(E) for a in range(b)]  # canonical order
G = len(PAIRS)  # 28
TCAP = 128  # hard per-(core, group) tile capacity (PE partition limit)
KCH = D // 128  # 4 contraction chunks

LAST_EXEC_TIME_NS = None  # set by kernel() when tracing is active

_cache = {}


def _build_bass(gcaps, pair_order):
    """Build the SPMD program for per-group capacities gcaps (len G).

    gcaps is derived from the actual routing counts (max over cores), so
    padding is near zero; pair_order[i] is the PAIRS entry computed by
    tile i. Tiles are size-sorted by the caller: matmuls whose stationary
    operand changes shape between issues pay a ~47ns PE reconfiguration,
    so equal-M tiles run consecutively. One 128-row-max tile per group: 8
    accumulating f32r matmuls into two PSUM banks + gate-weighted
    combine.
    """
    f32 = mybir.dt.float32
    f32r = mybir.dt.float32r
    rows = int(sum(gcaps))
    offs = np.concatenate([[0], np.cumsum(gcaps)]).astype(int)
    nc = bass.Bass()
    # xh: [p, (kc, col)] -> per-kc contiguous column ranges
    # wdma: [p, (e, kc, dout)] -> 8KB DMA runs, one DMA per expert
    xh = nc.declare_dram_parameter("xh", [128, rows * KCH], f32r, isOutput=False)
    pwa = nc.declare_dram_parameter("pwa", [TCAP, G * 2], f32, isOutput=False)
    wdma = nc.declare_dram_parameter(
        "wdma", [128, E * KCH * D], f32r, isOutput=False
    )
    y = nc.declare_dram_parameter("y", [rows, D], f32, isOutput=True)

    with TileContext(nc) as tc:
        with (
            tc.tile_pool(name="const", bufs=1) as cpool,
            tc.tile_pool(name="vpool", bufs=3) as vpool,
            tc.tile_pool(name="opool", bufs=3) as opool,
            tc.tile_pool(name="psum", bufs=3, space="PSUM") as pspool,
        ):
            pw_all = cpool.tile([TCAP, G * 2], f32)
            xsb = [
                cpool.tile([128, rows], f32r, tag=f"x{kc}", name=f"xsb{kc}")
                for kc in range(KCH)
            ]
            w_tiles = [
                cpool.tile([128, KCH * D], f32r, tag=f"w{e}", name=f"wsb{e}")
                for e in range(E)
            ]

            def load_w(e):
                nc.sync.dma_start(
                    w_tiles[e][:], wdma[:, e * KCH * D : (e + 1) * KCH * D]
                )

            def load_w_kc(e, kc):
                nc.sync.dma_start(
                    w_tiles[e][:, kc * D : (kc + 1) * D],
                    wdma[:, (e * KCH + kc) * D : (e * KCH + kc + 1) * D],
                )

            def load_x_cols(lo, hi):
                for kc in range(KCH):
                    nc.sync.dma_start(
                        xsb[kc][:, lo:hi],
                        xh[:, kc * rows + lo : kc * rows + hi],
                    )

            # loads merged in first-use order: for each tile, the experts
            # it introduces and the x-column block covering it must have
            # been issued; emit W loads and 4-tile x blocks sorted by the
            # tile that first needs them.
            e0, e1 = pair_order[0]
            load_w_kc(e0, 0)

            # PE warmup: the HAM clock gate defaults to 1.2 GHz and only
            # releases after ~3.4us of sustained PE activity; the first PE
            # instruction also pays the cold IRAM fetch. Run dummy matmuls
            # against the first-loaded W chunk during the DMA window so
            # the real matmuls start warm.
            warm_ps = pspool.tile([128, D], f32, tag="warm", bufs=1)
            for _ in range(12):
                nc.tensor.matmul(
                    warm_ps[0:64, :],
                    w_tiles[e0][:, 0:64],
                    w_tiles[e0][:, 0:D],
                    start=True,
                    stop=True,
                )

            load_w_kc(e1, 0)
            for kc in range(KCH):
                nc.sync.dma_start(
                    xsb[kc][:, 0 : offs[1]],
                    xh[:, kc * rows : kc * rows + offs[1]],
                )
            for kc in range(1, KCH):
                load_w_kc(e0, kc)
                load_w_kc(e1, kc)
            nc.sync.dma_start(pw_all[:], pwa[:, :])

            events = []  # (first_use_tile, priority, kind, arg)
            seen = {e0, e1}
            for t, pr in enumerate(pair_order):
                for e in pr:
                    if e not in seen:
                        seen.add(e)
                        events.append((t, 0, "w", e))
            nblk = (G + 3) // 4
            for k in range(nblk):
                t0 = 4 * k
                if t0 == 0:
                    lo, hi = int(offs[1]), int(offs[4])
                else:
                    lo, hi = int(offs[t0]), int(offs[min(t0 + 4, G)])
                # block k must be resident one tile-group early
                events.append((max(t0 - 2, 0), 1, "x", (lo, hi)))
            events.sort()
            for _, _, kind, arg in events:
                if kind == "w":
                    load_w(arg)
                else:
                    load_x_cols(*arg)

            for g, (a, b) in enumerate(pair_order):
                m = int(gcaps[g])
                lo, hi = int(offs[g]), int(offs[g] + m)
                pa = pspool.tile([TCAP, D], f32, tag="pa")
                pb = pspool.tile([TCAP, D], f32, tag="pb")
                for kc in range(KCH):
                    # both experts consume the same stationary x chunk
                    # back-to-back so walrus's ldw-opt can skip the reload
                    nc.tensor.matmul(
                        pa[0:m, :],
                        xsb[kc][:, lo:hi],
                        w_tiles[a][:, kc * D : (kc + 1) * D],
                        start=(kc == 0),
                        stop=(kc == KCH - 1),
                    )
                    nc.tensor.matmul(
                        pb[0:m, :],
                        xsb[kc][:, lo:hi],
                        w_tiles[b][:, kc * D : (kc + 1) * D],
                        start=(kc == 0),
                        stop=(kc == KCH - 1),
                    )
                # combine: out = pa*w_lo + pb*w_hi, split across ACT and DVE
                tmp = vpool.tile([TCAP, D], f32)
                nc.scalar.activation(
                    tmp[0:m, :],
                    pb[0:m, :],
                    mybir.ActivationFunctionType.Copy,
                    scale=pw_all[0:m, 2 * g + 1 : 2 * g + 2],
                )
                o = opool.tile([TCAP, D], f32)
                nc.vector.scalar_tensor_tensor(
                    o[0:m, :],
                    pa[0:m, :],
                    pw_all[0:m, 2 * g : 2 * g + 1],
                    tmp[0:m, :],
                    mybir.AluOpType.mult,
                    mybir.AluOpType.add,
                )
                nc.gpsimd.dma_start(y[lo:hi, :], o[0:m, :])
    _split_multi_waits(nc)
    return nc


def _assign(indices, probabilities, cap=TCAP):
    """Build per-core row assignments for every (token, gate) pair.

    Returns rows[c] = list of (token, group, w_lo, w_hi). Normal path:
    each token appears exactly once (both its gates land in the group of
    its expert pair). Groups whose per-core share exceeds TCAP, and
    duplicate-expert tokens, fall back to two single-gate rows placed in
    the least-loaded groups containing that expert.
    """
    gid = {p: g for g, p in enumerate(PAIRS)}
    groups_of = [[g for g, pr in enumerate(PAIRS) if e in pr] for e in range(E)]
    idx0, idx1 = indices[:, 0].astype(np.int64), indices[:, 1].astype(np.int64)
    p0, p1 = probabilities[:, 0], probabilities[:, 1]
    lo = np.minimum(idx0, idx1)
    hi = np.maximum(idx0, idx1)
    w_lo = np.where(idx0 <= idx1, p0, p1)
    w_hi = np.where(idx0 <= idx1, p1, p0)

    entries = [[] for _ in range(G)]  # group -> list of (token, w_lo, w_hi)
    singles = []  # (token, expert, weight) fallback entries
    dup = lo == hi
    for n in np.nonzero(dup)[0]:
        singles.append((int(n), int(lo[n]), float(p0[n] + p1[n])))
    ok = np.nonzero(~dup)[0]
    gids = np.array([gid[(int(a), int(b))] for a, b in zip(lo[ok], hi[ok])])
    for g in range(G):
        for n in ok[gids == g]:
            entries[g].append((int(n), float(w_lo[n]), float(w_hi[n])))

    rows = [[] for _ in range(NCORES)]  # core -> (token, group, wl, wh)
    used = np.zeros((NCORES, G), np.int64)
    for g in range(G):
        for j, (n, wl, wh) in enumerate(entries[g]):
            c = j % NCORES
            if used[c, g] < cap:
                rows[c].append((n, g, wl, wh))
                used[c, g] += 1
            else:
                a, b = PAIRS[g]
                singles.append((n, a, wl))
                singles.append((n, b, wh))
    for n, e, w in singles:
        # least-loaded (core, group) slot among groups containing e
        cands = [
            (used[c, g], c, g)
            for c in range(NCORES)
            for g in groups_of[e]
            if used[c, g] < cap
        ]
        assert cands, "no capacity left for fallback entry"
        _, c, g = min(cands)
        a, b = PAIRS[g]
        wl, wh = (w, 0.0) if e == a else (0.0, w)
        rows[c].append((n, g, wl, wh))
        used[c, g] += 1
    return rows, used


def kernel(input_batch, probabilities, indices, W, b, **_unused):
    global LAST_EXEC_TIME_NS
    x = np.ascontiguousarray(np.asarray(input_batch, dtype=np.float32))
    p = np.ascontiguousarray(np.asarray(probabilities, dtype=np.float32))
    idx = np.asarray(indices)
    Wf = np.ascontiguousarray(np.asarray(W, dtype=np.float32))
    bf = np.asarray(b, dtype=np.float32)
    assert x.shape == (N, D) and p.shape == (N, TOPK)
    assert idx.shape == (N, TOPK) and Wf.shape == (E, D, D)

    import os as _os
    _fix = int(_os.environ.get("MOE_FIXED_GCAP", "0"))
    rows, used = _assign(idx, p, cap=_fix if _fix else TCAP)
    # per-group M = max per-core occupancy quantized to {96, 128}: matmul
    # cost is free-dim-bound (M-independent), but issuing matmuls whose
    # stationary shape differs from the previous one costs ~47ns each, so
    # keep shapes uniform within two classes (single transition). Ranges
    # stay 32-aligned for DMA descriptor efficiency.
    occ = np.maximum(used.max(axis=0), 1)
    gcaps = np.where(occ <= 96, 96, TCAP).astype(np.int64)
    if _fix:
        gcaps = np.full(G, _fix, np.int64)
    # order: 96-class tiles first in triangle order (experts stream in
    # gradually), then the 128-class (chain pairs) once all weights are
    # resident
    order = np.array(
        sorted(range(G), key=lambda g: (int(gcaps[g]), g)), dtype=np.int64
    )
    gcaps = gcaps[order]
    pair_order = [PAIRS[int(g)] for g in order]
    pos = np.empty(G, np.int64)  # original group id -> tile index
    pos[order] = np.arange(G)
    offs = np.concatenate([[0], np.cumsum(gcaps)]).astype(int)
    rows_total = int(offs[-1])

    # [p, (e, kc, dout)] layout; see _build_bass
    wdma = np.ascontiguousarray(
        Wf.reshape(E, KCH, 128, D).transpose(2, 0, 1, 3).reshape(128, E * KCH * D)
    )

    in_maps = []
    tok_maps = []
    for c in range(NCORES):
        x_rows = np.zeros((rows_total, D), np.float32)
        pw_arr = np.zeros((rows_total, 2), np.float32)
        tok_arr = np.full(rows_total, -1, np.int64)
        slot_used = np.zeros(G, np.int64)
        for n, g, wl, wh in rows[c]:
            t = int(pos[g])
            s = int(offs[t] + slot_used[t])
            slot_used[t] += 1
            x_rows[s] = x[n]
            pw_arr[s, 0] = wl
            pw_arr[s, 1] = wh
            tok_arr[s] = n
        # xh: [p, (kc, col)]; see _build_bass
        xh = (
            x_rows.reshape(rows_total, KCH, 128)
            .transpose(2, 1, 0)
            .reshape(128, KCH * rows_total)
        )
        pwa = np.zeros((TCAP, G * 2), np.float32)
        for t in range(G):
            m = int(gcaps[t])
            pwa[0:m, 2 * t : 2 * t + 2] = pw_arr[offs[t] : offs[t] + m]
        in_maps.append(
            {
                "xh": np.ascontiguousarray(xh),
                "pwa": pwa,
                "wdma": wdma,
            }
        )
        tok_maps.append(tok_arr)

    key = (tuple(int(v) for v in gcaps), tuple(pair_order))
    if _cache.get("key") != key:
        _cache["nc"] = _build_bass(gcaps, pair_order)
        _cache["key"] = key
    nc = _cache["nc"]

    res = run_bass_kernel_spmd(nc, in_maps, list(range(NCORES)))
    LAST_EXEC_TIME_NS = res.exec_time_ns

    out = np.zeros((N, D), np.float32)
    all_tok = np.concatenate(tok_maps)
    all_y = np.concatenate([res.results[c]["y"] for c in range(NCORES)], axis=0)
    valid = all_tok >= 0
    vt = all_tok[valid]
    counts = np.bincount(vt, minlength=N)
    if counts.max() <= 1:
        out[vt] = all_y[valid]
    else:
        np.add.at(out, vt, all_y[valid])

    if np.any(bf):
        # gate-weighted bias: out[n] += sum_k p[n,k] * b[idx[n,k]]
        mask = np.zeros((N, E), np.float32)
        np.add.at(mask, (np.arange(N)[:, None], idx.astype(np.int64)), p)
        out += mask @ bf

    total_loss = np.float32(0.0)
    return out, total_loss


# revision 27
# speedup vs baseline: 1.0291x; 1.0291x over previous
"""MoE (N=16384, D=512, E=8, top_k=2) on 8 trn2 NeuronCores.

Strategy: group tokens globally by their unordered expert pair {e_a, e_b}
(28 groups for E=8), shard every group evenly across the 8 cores (96 slots
per core-group segment). Each core runs an identical (SPMD) program: 28
tiles of 96 tokens, each doing 8 accumulating float32r matmuls (2 experts
x 4 K-chunks, moving free dim 512) into two PSUM banks, then a
per-partition gate-weighted combine split across the Scalar and Vector
engines. All routing data-dependence lives in the host-side input
arrangement; the device program is fixed.
"""

import numpy as np

# ---------------------------------------------------------------------------
# The walrus build in this image accepts at most ONE sync-wait command per
# instruction, while Tile's semaphore assignment attaches several (DMA WAR +
# producer sems, and the kernel-tail drain waits on every live proc). Post-
# pass over the finished BIR: any instruction carrying more than one wait is
# preceded by same-engine nops that each take one wait. The engine executes
# its queue in order, so semantics are unchanged.
import bass_rust

_MAX_WAITS = 1


def _split_multi_waits(nc):
    for f in nc.m.functions:
        for blk in f.blocks:
            insts = blk.instructions
            k = 0
            while k < len(insts):
                inst = insts[k]
                si = getattr(inst, "sync_info", None)
                if si is not None and si.on_wait and len(si.on_wait) > _MAX_WAITS:
                    waits = list(si.on_wait)
                    keep = waits[-_MAX_WAITS:]
                    extra = waits[:-_MAX_WAITS]
                    inst.sync_info = bass_rust.SyncInfo(
                        on_wait=keep, on_update=list(si.on_update)
                    )
                    for j, i0 in enumerate(range(0, len(extra), _MAX_WAITS)):
                        nop = bass_rust.InstNoOp(
                            name=f"{inst.name}-wsplit{j}", ins=[], outs=[]
                        )
                        nop.engine = inst.engine
                        nop.sync_info = bass_rust.SyncInfo(
                            on_wait=extra[i0 : i0 + _MAX_WAITS], on_update=[]
                        )
                        insts.insert(k, nop)
                        k += 1
                k += 1
# ---------------------------------------------------------------------------

# Re-enable walrus's LDWEIGHTS dedup/pipelining pass: fp32r matmuls self-load
# their stationary operand, which would otherwise serialize with every
# matmul stream.
import concourse.bass_utils as _bu

if not getattr(_bu, "_ldw_opt_patched", False):
    _orig_run_command = _bu.run_command

    def _run_command_ldw(cmd, **kw):
        cmd = [
            "--enable-ldw-opt=true" if c == "--enable-ldw-opt=false" else c
            for c in cmd
        ]
        return _orig_run_command(cmd, **kw)

    _bu.run_command = _run_command_ldw
    _bu._ldw_opt_patched = True

import concourse.bass as bass
import concourse.mybir as mybir
from concourse.tile import TileContext
from concourse.bass_utils import run_bass_kernel_spmd

N, D, E, TOPK = 16384, 512, 8, 2
NCORES = 8
# triangle order: pair {a,b} (a<b) sorted by max expert, so expert m is
# first needed late and its weights can stream in during compute
PAIRS = [(a, b) for b in range(E) for a in range(b)]  # canonical order
G = len(PAIRS)  # 28
TCAP = 128  # hard per-(core, group) tile capacity (PE partition limit)
KCH = D // 128  # 4 contraction chunks

LAST_EXEC_TIME_NS = None  # set by kernel() when tracing is active

_cache = {}


def _build_bass(gcaps, pair_order):
    """Build the SPMD program for per-group capacities gcaps (len G).

    gcaps is derived from the actual routing counts (max over cores), so
    padding is near zero; pair_order[i] is the PAIRS entry computed by
    tile i. Tiles are size-sorted by the caller: matmuls whose stationary
    operand changes shape between issues pay a ~47ns PE reconfiguration,
    so equal-M tiles run consecutively. One 128-row-max tile per group: 8
    accumulating f32r matmuls into two PSUM banks + gate-weighted
    combine.
    """
    f32 = mybir.dt.float32
    f32r = mybir.dt.float32r
    rows = int(sum(gcaps))
    offs = np.concatenate([[0], np.cumsum(gcaps)]).astype(int)
    nc = bass.Bass()
    # xh: [p, (kc, col)] -> per-kc contiguous column ranges
    # wdma: [p, (e, kc, dout)] -> 8KB DMA runs, one DMA per expert
    xh = nc.declare_dram_parameter("xh", [128, rows * KCH], f32r, isOutput=False)
    pwa = nc.declare_dram_parameter("pwa", [TCAP, G * 2], f32, isOutput=False)
    wdma = nc.declare_dram_parameter(
        "wdma", [128, E * KCH * D], f32r, isOutput=False
    )
    y = nc.declare_dram_parameter("y", [rows, D], f32, isOutput=True)

    with TileContext(nc) as tc:
        with (
            tc.tile_pool(name="const", bufs=1) as cpool,
            tc.tile_pool(name="vpool", bufs=3) as vpool,
            tc.tile_pool(name="opool", bufs=3) as opool,
            tc.tile_pool(name="psum", bufs=3, space="PSUM") as pspool,
        ):
            pw_all = cpool.tile([TCAP, G * 2], f32)
            xsb = [
                cpool.tile([128, rows], f32r, tag=f"x{kc}", name=f"xsb{kc}")
                for kc in range(KCH)
            ]
            w_tiles = [
                cpool.tile([128, KCH * D], f32r, tag=f"w{e}", name=f"wsb{e}")
                for e in range(E)
            ]

            def load_w(e):
                nc.sync.dma_start(
                    w_tiles[e][:], wdma[:, e * KCH * D : (e + 1) * KCH * D]
                )

            def load_w_kc(e, kc):
                nc.sync.dma_start(
                    w_tiles[e][:, kc * D : (kc + 1) * D],
                    wdma[:, (e * KCH + kc) * D : (e * KCH + kc + 1) * D],
                )

            def load_x_cols(lo, hi):
                for kc in range(KCH):
                    nc.sync.dma_start(
                        xsb[kc][:, lo:hi],
                        xh[:, kc * rows + lo : kc * rows + hi],
                    )

            # loads merged in first-use order: for each tile, the experts
            # it introduces and the x-column block covering it must have
            # been issued; emit W loads and 4-tile x blocks sorted by the
            # tile that first needs them.
            e0, e1 = pair_order[0]
            load_w_kc(e0, 0)

            # PE warmup: the HAM clock gate defaults to 1.2 GHz and only
            # releases after ~3.4us of sustained PE activity; the first PE
            # instruction also pays the cold IRAM fetch. Run dummy matmuls
            # against the first-loaded W chunk during the DMA window so
            # the real matmuls start warm.
            warm_ps = pspool.tile([128, D], f32, tag="warm", bufs=1)
            for _ in range(8):
                nc.tensor.matmul(
                    warm_ps[0:64, :],
                    w_tiles[e0][:, 0:64],
                    w_tiles[e0][:, 0:D],
                    start=True,
                    stop=True,
                )

            load_w_kc(e1, 0)
            for kc in range(KCH):
                nc.sync.dma_start(
                    xsb[kc][:, 0 : offs[1]],
                    xh[:, kc * rows : kc * rows + offs[1]],
                )
            for kc in range(1, KCH):
                load_w_kc(e0, kc)
                load_w_kc(e1, kc)
            nc.sync.dma_start(pw_all[:], pwa[:, :])

            events = []  # (first_use_tile, priority, kind, arg)
            seen = {e0, e1}
            for t, pr in enumerate(pair_order):
                for e in pr:
                    if e not in seen:
                        seen.add(e)
                        # lead the first use by 4 tiles: a matmul whose
                        # weight sem-wait is unresolved also loses the
                        # LDWEIGHTS pull-ahead (427ns instead of 234ns)
                        events.append((max(t - 4, 0), 0, "w", e))
            nblk = (G + 3) // 4
            for k in range(nblk):
                t0 = 4 * k
                if t0 == 0:
                    lo, hi = int(offs[1]), int(offs[4])
                else:
                    lo, hi = int(offs[t0]), int(offs[min(t0 + 4, G)])
                events.append((max(t0 - 3, 0), 1, "x", (lo, hi)))
            events.sort()
            for _, _, kind, arg in events:
                if kind == "w":
                    load_w(arg)
                else:
                    load_x_cols(*arg)

            for g, (a, b) in enumerate(pair_order):
                m = int(gcaps[g])
                lo, hi = int(offs[g]), int(offs[g] + m)
                pa = pspool.tile([TCAP, D], f32, tag="pa")
                pb = pspool.tile([TCAP, D], f32, tag="pb")
                for kc in range(KCH):
                    # both experts consume the same stationary x chunk
                    # back-to-back so walrus's ldw-opt can skip the reload
                    nc.tensor.matmul(
                        pa[0:m, :],
                        xsb[kc][:, lo:hi],
                        w_tiles[a][:, kc * D : (kc + 1) * D],
                        start=(kc == 0),
                        stop=(kc == KCH - 1),
                    )
                    nc.tensor.matmul(
                        pb[0:m, :],
                        xsb[kc][:, lo:hi],
                        w_tiles[b][:, kc * D : (kc + 1) * D],
                        start=(kc == 0),
                        stop=(kc == KCH - 1),
                    )
                # combine: out = pa*w_lo + pb*w_hi, split across ACT and DVE
                tmp = vpool.tile([TCAP, D], f32)
                nc.scalar.activation(
                    tmp[0:m, :],
                    pb[0:m, :],
                    mybir.ActivationFunctionType.Copy,
                    scale=pw_all[0:m, 2 * g + 1 : 2 * g + 2],
                )
                o = opool.tile([TCAP, D], f32)
                nc.vector.scalar_tensor_tensor(
                    o[0:m, :],
                    pa[0:m, :],
                    pw_all[0:m, 2 * g : 2 * g + 1],
                    tmp[0:m, :],
                    mybir.AluOpType.mult,
                    mybir.AluOpType.add,
                )
                nc.gpsimd.dma_start(y[lo:hi, :], o[0:m, :])
    _split_multi_waits(nc)
    return nc


def _assign(indices, probabilities, cap=TCAP):
    """Build per-core row assignments for every (token, gate) pair.

    Returns rows[c] = list of (token, group, w_lo, w_hi). Normal path:
    each token appears exactly once (both its gates land in the group of
    its expert pair). Groups whose per-core share exceeds TCAP, and
    duplicate-expert tokens, fall back to two single-gate rows placed in
    the least-loaded groups containing that expert.
    """
    gid = {p: g for g, p in enumerate(PAIRS)}
    groups_of = [[g for g, pr in enumerate(PAIRS) if e in pr] for e in range(E)]
    idx0, idx1 = indices[:, 0].astype(np.int64), indices[:, 1].astype(np.int64)
    p0, p1 = probabilities[:, 0], probabilities[:, 1]
    lo = np.minimum(idx0, idx1)
    hi = np.maximum(idx0, idx1)
    w_lo = np.where(idx0 <= idx1, p0, p1)
    w_hi = np.where(idx0 <= idx1, p1, p0)

    entries = [[] for _ in range(G)]  # group -> list of (token, w_lo, w_hi)
    singles = []  # (token, expert, weight) fallback entries
    dup = lo == hi
    for n in np.nonzero(dup)[0]:
        singles.append((int(n), int(lo[n]), float(p0[n] + p1[n])))
    ok = np.nonzero(~dup)[0]
    gids = np.array([gid[(int(a), int(b))] for a, b in zip(lo[ok], hi[ok])])
    for g in range(G):
        for n in ok[gids == g]:
            entries[g].append((int(n), float(w_lo[n]), float(w_hi[n])))

    rows = [[] for _ in range(NCORES)]  # core -> (token, group, wl, wh)
    used = np.zeros((NCORES, G), np.int64)
    for g in range(G):
        for j, (n, wl, wh) in enumerate(entries[g]):
            c = j % NCORES
            if used[c, g] < cap:
                rows[c].append((n, g, wl, wh))
                used[c, g] += 1
            else:
                a, b = PAIRS[g]
                singles.append((n, a, wl))
                singles.append((n, b, wh))
    for n, e, w in singles:
        # least-loaded (core, group) slot among groups containing e
        cands = [
            (used[c, g], c, g)
            for c in range(NCORES)
            for g in groups_of[e]
            if used[c, g] < cap
        ]
        assert cands, "no capacity left for fallback entry"
        _, c, g = min(cands)
        a, b = PAIRS[g]
        wl, wh = (w, 0.0) if e == a else (0.0, w)
        rows[c].append((n, g, wl, wh))
        used[c, g] += 1
    return rows, used


def kernel(input_batch, probabilities, indices, W, b, **_unused):
    global LAST_EXEC_TIME_NS
    x = np.ascontiguousarray(np.asarray(input_batch, dtype=np.float32))
    p = np.ascontiguousarray(np.asarray(probabilities, dtype=np.float32))
    idx = np.asarray(indices)
    Wf = np.ascontiguousarray(np.asarray(W, dtype=np.float32))
    bf = np.asarray(b, dtype=np.float32)
    assert x.shape == (N, D) and p.shape == (N, TOPK)
    assert idx.shape == (N, TOPK) and Wf.shape == (E, D, D)

    import os as _os
    _fix = int(_os.environ.get("MOE_FIXED_GCAP", "0"))
    rows, used = _assign(idx, p, cap=_fix if _fix else TCAP)
    # per-group M = max per-core occupancy quantized to {96, 128}: matmul
    # cost is free-dim-bound (M-independent), but issuing matmuls whose
    # stationary shape differs from the previous one costs ~47ns each, so
    # keep shapes uniform within two classes (single transition). Ranges
    # stay 32-aligned for DMA descriptor efficiency.
    occ = np.maximum(used.max(axis=0), 1)
    gcaps = np.where(occ <= 96, 96, TCAP).astype(np.int64)
    if _fix:
        gcaps = np.full(G, _fix, np.int64)
    # order: 96-class tiles first in triangle order (experts stream in
    # gradually), then the 128-class (chain pairs) once all weights are
    # resident
    order = np.array(
        sorted(range(G), key=lambda g: (int(gcaps[g]), g)), dtype=np.int64
    )
    gcaps = gcaps[order]
    pair_order = [PAIRS[int(g)] for g in order]
    pos = np.empty(G, np.int64)  # original group id -> tile index
    pos[order] = np.arange(G)
    offs = np.concatenate([[0], np.cumsum(gcaps)]).astype(int)
    rows_total = int(offs[-1])

    # [p, (e, kc, dout)] layout; see _build_bass
    wdma = np.ascontiguousarray(
        Wf.reshape(E, KCH, 128, D).transpose(2, 0, 1, 3).reshape(128, E * KCH * D)
    )

    in_maps = []
    tok_maps = []
    for c in range(NCORES):
        x_rows = np.zeros((rows_total, D), np.float32)
        pw_arr = np.zeros((rows_total, 2), np.float32)
        tok_arr = np.full(rows_total, -1, np.int64)
        slot_used = np.zeros(G, np.int64)
        for n, g, wl, wh in rows[c]:
            t = int(pos[g])
            s = int(offs[t] + slot_used[t])
            slot_used[t] += 1
            x_rows[s] = x[n]
            pw_arr[s, 0] = wl
            pw_arr[s, 1] = wh
            tok_arr[s] = n
        # xh: [p, (kc, col)]; see _build_bass
        xh = (
            x_rows.reshape(rows_total, KCH, 128)
            .transpose(2, 1, 0)
            .reshape(128, KCH * rows_total)
        )
        pwa = np.zeros((TCAP, G * 2), np.float32)
        for t in range(G):
            m = int(gcaps[t])
            pwa[0:m, 2 * t : 2 * t + 2] = pw_arr[offs[t] : offs[t] + m]
        in_maps.append(
            {
                "xh": np.ascontiguousarray(xh),
                "pwa": pwa,
                "wdma": wdma,
            }
        )
        tok_maps.append(tok_arr)

    key = (tuple(int(v) for v in gcaps), tuple(pair_order))
    if _cache.get("key") != key:
        _cache["nc"] = _build_bass(gcaps, pair_order)
        _cache["key"] = key
    nc = _cache["nc"]

    res = run_bass_kernel_spmd(nc, in_maps, list(range(NCORES)))
    LAST_EXEC_TIME_NS = res.exec_time_ns

    out = np.zeros((N, D), np.float32)
    all_tok = np.concatenate(tok_maps)
    all_y = np.concatenate([res.results[c]["y"] for c in range(NCORES)], axis=0)
    valid = all_tok >= 0
    vt = all_tok[valid]
    counts = np.bincount(vt, minlength=N)
    if counts.max() <= 1:
        out[vt] = all_y[valid]
    else:
        np.add.at(out, vt, all_y[valid])

    if np.any(bf):
        # gate-weighted bias: out[n] += sum_k p[n,k] * b[idx[n,k]]
        mask = np.zeros((N, E), np.float32)
        np.add.at(mask, (np.arange(N)[:, None], idx.astype(np.int64)), p)
        out += mask @ bf

    total_loss = np.float32(0.0)
    return out, total_loss


# revision 28
# speedup vs baseline: 1.0354x; 1.0060x over previous
"""MoE (N=16384, D=512, E=8, top_k=2) on 8 trn2 NeuronCores.

Strategy: group tokens globally by their unordered expert pair {e_a, e_b}
(28 groups for E=8), shard every group evenly across the 8 cores (96 slots
per core-group segment). Each core runs an identical (SPMD) program: 28
tiles of 96 tokens, each doing 8 accumulating float32r matmuls (2 experts
x 4 K-chunks, moving free dim 512) into two PSUM banks, then a
per-partition gate-weighted combine split across the Scalar and Vector
engines. All routing data-dependence lives in the host-side input
arrangement; the device program is fixed.
"""

import numpy as np

# ---------------------------------------------------------------------------
# The walrus build in this image accepts at most ONE sync-wait command per
# instruction, while Tile's semaphore assignment attaches several (DMA WAR +
# producer sems, and the kernel-tail drain waits on every live proc). Post-
# pass over the finished BIR: any instruction carrying more than one wait is
# preceded by same-engine nops that each take one wait. The engine executes
# its queue in order, so semantics are unchanged.
import bass_rust

_MAX_WAITS = 1


def _split_multi_waits(nc):
    import concourse.mybir as _mybir

    # the kernel-tail drain's waits can run on ANY engine: the all-engine
    # barrier that follows joins every queue, so spreading them 5-wide
    # turns a ~27-step serial wait chain into ~6 steps per engine
    spread = [
        _mybir.EngineType.SP,
        _mybir.EngineType.PE,
        _mybir.EngineType.DVE,
        _mybir.EngineType.Activation,
        _mybir.EngineType.Pool,
    ]
    for f in nc.m.functions:
        for blk in f.blocks:
            insts = blk.instructions
            k = 0
            while k < len(insts):
                inst = insts[k]
                si = getattr(inst, "sync_info", None)
                if si is not None and si.on_wait and len(si.on_wait) > _MAX_WAITS:
                    is_drain = type(inst).__name__ == "InstDrain"
                    waits = list(si.on_wait)
                    keep = waits[-_MAX_WAITS:]
                    extra = waits[:-_MAX_WAITS]
                    inst.sync_info = bass_rust.SyncInfo(
                        on_wait=keep, on_update=list(si.on_update)
                    )
                    for j, i0 in enumerate(range(0, len(extra), _MAX_WAITS)):
                        nop = bass_rust.InstNoOp(
                            name=f"{inst.name}-wsplit{j}", ins=[], outs=[]
                        )
                        nop.engine = (
                            spread[j % len(spread)] if is_drain else inst.engine
                        )
                        nop.sync_info = bass_rust.SyncInfo(
                            on_wait=extra[i0 : i0 + _MAX_WAITS], on_update=[]
                        )
                        insts.insert(k, nop)
                        k += 1
                k += 1
# ---------------------------------------------------------------------------

# Re-enable walrus's LDWEIGHTS dedup/pipelining pass: fp32r matmuls self-load
# their stationary operand, which would otherwise serialize with every
# matmul stream.
import concourse.bass_utils as _bu

if not getattr(_bu, "_ldw_opt_patched", False):
    _orig_run_command = _bu.run_command

    def _run_command_ldw(cmd, **kw):
        cmd = [
            "--enable-ldw-opt=true" if c == "--enable-ldw-opt=false" else c
            for c in cmd
        ]
        return _orig_run_command(cmd, **kw)

    _bu.run_command = _run_command_ldw
    _bu._ldw_opt_patched = True

import concourse.bass as bass
import concourse.mybir as mybir
from concourse.tile import TileContext
from concourse.bass_utils import run_bass_kernel_spmd

N, D, E, TOPK = 16384, 512, 8, 2
NCORES = 8
# triangle order: pair {a,b} (a<b) sorted by max expert, so expert m is
# first needed late and its weights can stream in during compute
PAIRS = [(a, b) for b in range(E) for a in range(b)]  # canonical order
G = len(PAIRS)  # 28
TCAP = 128  # hard per-(core, group) tile capacity (PE partition limit)
KCH = D // 128  # 4 contraction chunks

LAST_EXEC_TIME_NS = None  # set by kernel() when tracing is active

_cache = {}


def _build_bass(gcaps, pair_order):
    """Build the SPMD program for per-group capacities gcaps (len G).

    gcaps is derived from the actual routing counts (max over cores), so
    padding is near zero; pair_order[i] is the PAIRS entry computed by
    tile i. Tiles are size-sorted by the caller: matmuls whose stationary
    operand changes shape between issues pay a ~47ns PE reconfiguration,
    so equal-M tiles run consecutively. One 128-row-max tile per group: 8
    accumulating f32r matmuls into two PSUM banks + gate-weighted
    combine.
    """
    f32 = mybir.dt.float32
    f32r = mybir.dt.float32r
    rows = int(sum(gcaps))
    offs = np.concatenate([[0], np.cumsum(gcaps)]).astype(int)
    nc = bass.Bass()
    # xh: [p, (kc, col)] -> per-kc contiguous column ranges
    # wdma: [p, (e, kc, dout)] -> 8KB DMA runs, one DMA per expert
    xh = nc.declare_dram_parameter("xh", [128, rows * KCH], f32r, isOutput=False)
    pwa = nc.declare_dram_parameter("pwa", [TCAP, G * 2], f32, isOutput=False)
    wdma = nc.declare_dram_parameter(
        "wdma", [128, E * KCH * D], f32r, isOutput=False
    )
    y = nc.declare_dram_parameter("y", [rows, D], f32, isOutput=True)

    with TileContext(nc) as tc:
        with (
            tc.tile_pool(name="const", bufs=1) as cpool,
            tc.tile_pool(name="vpool", bufs=3) as vpool,
            tc.tile_pool(name="opool", bufs=3) as opool,
            tc.tile_pool(name="psum", bufs=3, space="PSUM") as pspool,
        ):
            pw_all = cpool.tile([TCAP, G * 2], f32)
            xsb = [
                cpool.tile([128, rows], f32r, tag=f"x{kc}", name=f"xsb{kc}")
                for kc in range(KCH)
            ]
            w_tiles = [
                cpool.tile([128, KCH * D], f32r, tag=f"w{e}", name=f"wsb{e}")
                for e in range(E)
            ]

            def load_w(e):
                nc.sync.dma_start(
                    w_tiles[e][:], wdma[:, e * KCH * D : (e + 1) * KCH * D]
                )

            def load_w_kc(e, kc):
                nc.sync.dma_start(
                    w_tiles[e][:, kc * D : (kc + 1) * D],
                    wdma[:, (e * KCH + kc) * D : (e * KCH + kc + 1) * D],
                )

            def load_x_cols(lo, hi):
                for kc in range(KCH):
                    nc.sync.dma_start(
                        xsb[kc][:, lo:hi],
                        xh[:, kc * rows + lo : kc * rows + hi],
                    )

            # loads merged in first-use order: for each tile, the experts
            # it introduces and the x-column block covering it must have
            # been issued; emit W loads and 4-tile x blocks sorted by the
            # tile that first needs them.
            e0, e1 = pair_order[0]
            load_w_kc(e0, 0)

            # PE warmup: the HAM clock gate defaults to 1.2 GHz and only
            # releases after ~3.4us of sustained PE activity; the first PE
            # instruction also pays the cold IRAM fetch. Run dummy matmuls
            # against the first-loaded W chunk during the DMA window so
            # the real matmuls start warm.
            warm_ps = pspool.tile([128, D], f32, tag="warm", bufs=1)
            for _ in range(8):
                nc.tensor.matmul(
                    warm_ps[0:64, :],
                    w_tiles[e0][:, 0:64],
                    w_tiles[e0][:, 0:D],
                    start=True,
                    stop=True,
                )

            load_w_kc(e1, 0)
            for kc in range(KCH):
                nc.sync.dma_start(
                    xsb[kc][:, 0 : offs[1]],
                    xh[:, kc * rows : kc * rows + offs[1]],
                )
            for kc in range(1, KCH):
                load_w_kc(e0, kc)
                load_w_kc(e1, kc)
            nc.sync.dma_start(pw_all[:], pwa[:, :])

            events = []  # (first_use_tile, priority, kind, arg)
            seen = {e0, e1}
            for t, pr in enumerate(pair_order):
                for e in pr:
                    if e not in seen:
                        seen.add(e)
                        # lead the first use by 4 tiles: a matmul whose
                        # weight sem-wait is unresolved also loses the
                        # LDWEIGHTS pull-ahead (427ns instead of 234ns)
                        events.append((max(t - 4, 0), 0, "w", e))
            nblk = (G + 3) // 4
            for k in range(nblk):
                t0 = 4 * k
                if t0 == 0:
                    lo, hi = int(offs[1]), int(offs[4])
                else:
                    lo, hi = int(offs[t0]), int(offs[min(t0 + 4, G)])
                events.append((max(t0 - 3, 0), 1, "x", (lo, hi)))
            events.sort()
            for _, _, kind, arg in events:
                if kind == "w":
                    load_w(arg)
                else:
                    load_x_cols(*arg)

            for g, (a, b) in enumerate(pair_order):
                m = int(gcaps[g])
                lo, hi = int(offs[g]), int(offs[g] + m)
                pa = pspool.tile([TCAP, D], f32, tag="pa")
                pb = pspool.tile([TCAP, D], f32, tag="pb")
                for kc in range(KCH):
                    # both experts consume the same stationary x chunk
                    # back-to-back so walrus's ldw-opt can skip the reload
                    nc.tensor.matmul(
                        pa[0:m, :],
                        xsb[kc][:, lo:hi],
                        w_tiles[a][:, kc * D : (kc + 1) * D],
                        start=(kc == 0),
                        stop=(kc == KCH - 1),
                    )
                    nc.tensor.matmul(
                        pb[0:m, :],
                        xsb[kc][:, lo:hi],
                        w_tiles[b][:, kc * D : (kc + 1) * D],
                        start=(kc == 0),
                        stop=(kc == KCH - 1),
                    )
                # combine: out = pa*w_lo + pb*w_hi, split across ACT and DVE
                tmp = vpool.tile([TCAP, D], f32)
                nc.scalar.activation(
                    tmp[0:m, :],
                    pb[0:m, :],
                    mybir.ActivationFunctionType.Copy,
                    scale=pw_all[0:m, 2 * g + 1 : 2 * g + 2],
                )
                o = opool.tile([TCAP, D], f32)
                nc.vector.scalar_tensor_tensor(
                    o[0:m, :],
                    pa[0:m, :],
                    pw_all[0:m, 2 * g : 2 * g + 1],
                    tmp[0:m, :],
                    mybir.AluOpType.mult,
                    mybir.AluOpType.add,
                )
                nc.gpsimd.dma_start(y[lo:hi, :], o[0:m, :])
    _split_multi_waits(nc)
    return nc


def _assign(indices, probabilities, cap=TCAP):
    """Build per-core row assignments for every (token, gate) pair.

    Returns rows[c] = list of (token, group, w_lo, w_hi). Normal path:
    each token appears exactly once (both its gates land in the group of
    its expert pair). Groups whose per-core share exceeds TCAP, and
    duplicate-expert tokens, fall back to two single-gate rows placed in
    the least-loaded groups containing that expert.
    """
    gid = {p: g for g, p in enumerate(PAIRS)}
    groups_of = [[g for g, pr in enumerate(PAIRS) if e in pr] for e in range(E)]
    idx0, idx1 = indices[:, 0].astype(np.int64), indices[:, 1].astype(np.int64)
    p0, p1 = probabilities[:, 0], probabilities[:, 1]
    lo = np.minimum(idx0, idx1)
    hi = np.maximum(idx0, idx1)
    w_lo = np.where(idx0 <= idx1, p0, p1)
    w_hi = np.where(idx0 <= idx1, p1, p0)

    entries = [[] for _ in range(G)]  # group -> list of (token, w_lo, w_hi)
    singles = []  # (token, expert, weight) fallback entries
    dup = lo == hi
    for n in np.nonzero(dup)[0]:
        singles.append((int(n), int(lo[n]), float(p0[n] + p1[n])))
    ok = np.nonzero(~dup)[0]
    gids = np.array([gid[(int(a), int(b))] for a, b in zip(lo[ok], hi[ok])])
    for g in range(G):
        for n in ok[gids == g]:
            entries[g].append((int(n), float(w_lo[n]), float(w_hi[n])))

    rows = [[] for _ in range(NCORES)]  # core -> (token, group, wl, wh)
    used = np.zeros((NCORES, G), np.int64)
    for g in range(G):
        for j, (n, wl, wh) in enumerate(entries[g]):
            c = j % NCORES
            if used[c, g] < cap:
                rows[c].append((n, g, wl, wh))
                used[c, g] += 1
            else:
                a, b = PAIRS[g]
                singles.append((n, a, wl))
                singles.append((n, b, wh))
    for n, e, w in singles:
        # least-loaded (core, group) slot among groups containing e
        cands = [
            (used[c, g], c, g)
            for c in range(NCORES)
            for g in groups_of[e]
            if used[c, g] < cap
        ]
        assert cands, "no capacity left for fallback entry"
        _, c, g = min(cands)
        a, b = PAIRS[g]
        wl, wh = (w, 0.0) if e == a else (0.0, w)
        rows[c].append((n, g, wl, wh))
        used[c, g] += 1
    return rows, used


def kernel(input_batch, probabilities, indices, W, b, **_unused):
    global LAST_EXEC_TIME_NS
    x = np.ascontiguousarray(np.asarray(input_batch, dtype=np.float32))
    p = np.ascontiguousarray(np.asarray(probabilities, dtype=np.float32))
    idx = np.asarray(indices)
    Wf = np.ascontiguousarray(np.asarray(W, dtype=np.float32))
    bf = np.asarray(b, dtype=np.float32)
    assert x.shape == (N, D) and p.shape == (N, TOPK)
    assert idx.shape == (N, TOPK) and Wf.shape == (E, D, D)

    import os as _os
    _fix = int(_os.environ.get("MOE_FIXED_GCAP", "0"))
    rows, used = _assign(idx, p, cap=_fix if _fix else TCAP)
    # per-group M = max per-core occupancy quantized to {96, 128}: matmul
    # cost is free-dim-bound (M-independent), but issuing matmuls whose
    # stationary shape differs from the previous one costs ~47ns each, so
    # keep shapes uniform within two classes (single transition). Ranges
    # stay 32-aligned for DMA descriptor efficiency.
    occ = np.maximum(used.max(axis=0), 1)
    gcaps = np.where(occ <= 96, 96, TCAP).astype(np.int64)
    if _fix:
        gcaps = np.full(G, _fix, np.int64)
    # order: 96-class tiles first in triangle order (experts stream in
    # gradually), then the 128-class (chain pairs) once all weights are
    # resident
    order = np.array(
        sorted(range(G), key=lambda g: (int(gcaps[g]), g)), dtype=np.int64
    )
    gcaps = gcaps[order]
    pair_order = [PAIRS[int(g)] for g in order]
    pos = np.empty(G, np.int64)  # original group id -> tile index
    pos[order] = np.arange(G)
    offs = np.concatenate([[0], np.cumsum(gcaps)]).astype(int)
    rows_total = int(offs[-1])

    # [p, (e, kc, dout)] layout; see _build_bass
    wdma = np.ascontiguousarray(
        Wf.reshape(E, KCH, 128, D).transpose(2, 0, 1, 3).reshape(128, E * KCH * D)
    )

    in_maps = []
    tok_maps = []
    for c in range(NCORES):
        x_rows = np.zeros((rows_total, D), np.float32)
        pw_arr = np.zeros((rows_total, 2), np.float32)
        tok_arr = np.full(rows_total, -1, np.int64)
        slot_used = np.zeros(G, np.int64)
        for n, g, wl, wh in rows[c]:
            t = int(pos[g])
            s = int(offs[t] + slot_used[t])
            slot_used[t] += 1
            x_rows[s] = x[n]
            pw_arr[s, 0] = wl
            pw_arr[s, 1] = wh
            tok_arr[s] = n
        # xh: [p, (kc, col)]; see _build_bass
        xh = (
            x_rows.reshape(rows_total, KCH, 128)
            .transpose(2, 1, 0)
            .reshape(128, KCH * rows_total)
        )
        pwa = np.zeros((TCAP, G * 2), np.float32)
        for t in range(G):
            m = int(gcaps[t])
            pwa[0:m, 2 * t : 2 * t + 2] = pw_arr[offs[t] : offs[t] + m]
        in_maps.append(
            {
                "xh": np.ascontiguousarray(xh),
                "pwa": pwa,
                "wdma": wdma,
            }
        )
        tok_maps.append(tok_arr)

    key = (tuple(int(v) for v in gcaps), tuple(pair_order))
    if _cache.get("key") != key:
        _cache["nc"] = _build_bass(gcaps, pair_order)
        _cache["key"] = key
    nc = _cache["nc"]

    res = run_bass_kernel_spmd(nc, in_maps, list(range(NCORES)))
    LAST_EXEC_TIME_NS = res.exec_time_ns

    out = np.zeros((N, D), np.float32)
    all_tok = np.concatenate(tok_maps)
    all_y = np.concatenate([res.results[c]["y"] for c in range(NCORES)], axis=0)
    valid = all_tok >= 0
    vt = all_tok[valid]
    counts = np.bincount(vt, minlength=N)
    if counts.max() <= 1:
        out[vt] = all_y[valid]
    else:
        np.add.at(out, vt, all_y[valid])

    if np.any(bf):
        # gate-weighted bias: out[n] += sum_k p[n,k] * b[idx[n,k]]
        mask = np.zeros((N, E), np.float32)
        np.add.at(mask, (np.arange(N)[:, None], idx.astype(np.int64)), p)
        out += mask @ bf

    total_loss = np.float32(0.0)
    return out, total_loss


# revision 29
# speedup vs baseline: 1.0836x; 1.0466x over previous
"""MoE (N=16384, D=512, E=8, top_k=2) on 8 trn2 NeuronCores.

Strategy: group tokens globally by their unordered expert pair {e_a, e_b}
(28 groups for E=8), shard every group evenly across the 8 cores (96 slots
per core-group segment). Each core runs an identical (SPMD) program: 28
tiles of 96 tokens, each doing 8 accumulating float32r matmuls (2 experts
x 4 K-chunks, moving free dim 512) into two PSUM banks, then a
per-partition gate-weighted combine split across the Scalar and Vector
engines. All routing data-dependence lives in the host-side input
arrangement; the device program is fixed.
"""

import numpy as np

# ---------------------------------------------------------------------------
# The walrus build in this image accepts at most ONE sync-wait command per
# instruction, while Tile's semaphore assignment attaches several (DMA WAR +
# producer sems, and the kernel-tail drain waits on every live proc). Post-
# pass over the finished BIR: any instruction carrying more than one wait is
# preceded by same-engine nops that each take one wait. The engine executes
# its queue in order, so semantics are unchanged.
import bass_rust

_MAX_WAITS = 1


def _split_multi_waits(nc):
    import concourse.mybir as _mybir

    # the kernel-tail drain's waits can run on ANY engine: the all-engine
    # barrier that follows joins every queue, so spreading them 5-wide
    # turns a ~27-step serial wait chain into ~6 steps per engine
    spread = [
        _mybir.EngineType.SP,
        _mybir.EngineType.PE,
        _mybir.EngineType.DVE,
        _mybir.EngineType.Activation,
        _mybir.EngineType.Pool,
    ]
    for f in nc.m.functions:
        for blk in f.blocks:
            insts = blk.instructions
            k = 0
            while k < len(insts):
                inst = insts[k]
                si = getattr(inst, "sync_info", None)
                if si is not None and si.on_wait and len(si.on_wait) > _MAX_WAITS:
                    is_drain = type(inst).__name__ == "InstDrain"
                    waits = list(si.on_wait)
                    keep = waits[-_MAX_WAITS:]
                    extra = waits[:-_MAX_WAITS]
                    inst.sync_info = bass_rust.SyncInfo(
                        on_wait=keep, on_update=list(si.on_update)
                    )
                    for j, i0 in enumerate(range(0, len(extra), _MAX_WAITS)):
                        nop = bass_rust.InstNoOp(
                            name=f"{inst.name}-wsplit{j}", ins=[], outs=[]
                        )
                        nop.engine = (
                            spread[j % len(spread)] if is_drain else inst.engine
                        )
                        nop.sync_info = bass_rust.SyncInfo(
                            on_wait=extra[i0 : i0 + _MAX_WAITS], on_update=[]
                        )
                        insts.insert(k, nop)
                        k += 1
                k += 1
# ---------------------------------------------------------------------------

# Re-enable walrus's LDWEIGHTS dedup/pipelining pass: fp32r matmuls self-load
# their stationary operand, which would otherwise serialize with every
# matmul stream.
import concourse.bass_utils as _bu

if not getattr(_bu, "_ldw_opt_patched", False):
    _orig_run_command = _bu.run_command

    def _run_command_ldw(cmd, **kw):
        cmd = [
            "--enable-ldw-opt=true" if c == "--enable-ldw-opt=false" else c
            for c in cmd
        ]
        return _orig_run_command(cmd, **kw)

    _bu.run_command = _run_command_ldw
    _bu._ldw_opt_patched = True

import concourse.bass as bass
import concourse.mybir as mybir
from concourse.tile import TileContext
from concourse.bass_utils import run_bass_kernel_spmd

N, D, E, TOPK = 16384, 512, 8, 2
NCORES = 8
# triangle order: pair {a,b} (a<b) sorted by max expert, so expert m is
# first needed late and its weights can stream in during compute
PAIRS = [(a, b) for b in range(E) for a in range(b)]  # canonical order
G = len(PAIRS)  # 28
TCAP = 128  # hard per-(core, group) tile capacity (PE partition limit)
KCH = D // 128  # 4 contraction chunks

LAST_EXEC_TIME_NS = None  # set by kernel() when tracing is active

_cache = {}


def _build_bass(gcaps, pair_order):
    """Build the SPMD program for per-group capacities gcaps (len G).

    gcaps is derived from the actual routing counts (max over cores), so
    padding is near zero; pair_order[i] is the PAIRS entry computed by
    tile i. Tiles are size-sorted by the caller: matmuls whose stationary
    operand changes shape between issues pay a ~47ns PE reconfiguration,
    so equal-M tiles run consecutively. One 128-row-max tile per group: 8
    accumulating f32r matmuls into two PSUM banks + gate-weighted
    combine.
    """
    f32 = mybir.dt.float32
    f32r = mybir.dt.float32r
    rows = int(sum(gcaps))
    offs = np.concatenate([[0], np.cumsum(gcaps)]).astype(int)
    nc = bass.Bass()
    # xh: [p, (kc, col)] -> per-kc contiguous column ranges
    # wdma: [p, (e, kc, dout)] -> 8KB DMA runs, one DMA per expert
    xh = nc.declare_dram_parameter("xh", [128, rows * KCH], f32r, isOutput=False)
    pwa = nc.declare_dram_parameter("pwa", [TCAP, G * 2], f32, isOutput=False)
    wdma = nc.declare_dram_parameter(
        "wdma", [128, E * KCH * D], f32r, isOutput=False
    )
    y = nc.declare_dram_parameter("y", [rows, D], f32, isOutput=True)

    with TileContext(nc) as tc:
        with (
            tc.tile_pool(name="const", bufs=1) as cpool,
            tc.tile_pool(name="vpool", bufs=3) as vpool,
            tc.tile_pool(name="opool", bufs=3) as opool,
            tc.tile_pool(name="psum", bufs=3, space="PSUM") as pspool,
        ):
            pw_all = cpool.tile([TCAP, G * 2], f32)
            xsb = [
                cpool.tile([128, rows], f32r, tag=f"x{kc}", name=f"xsb{kc}")
                for kc in range(KCH)
            ]
            w_tiles = [
                cpool.tile([128, KCH * D], f32r, tag=f"w{e}", name=f"wsb{e}")
                for e in range(E)
            ]

            def load_w(e):
                nc.sync.dma_start(
                    w_tiles[e][:], wdma[:, e * KCH * D : (e + 1) * KCH * D]
                )

            def load_w_kc(e, kc):
                nc.sync.dma_start(
                    w_tiles[e][:, kc * D : (kc + 1) * D],
                    wdma[:, (e * KCH + kc) * D : (e * KCH + kc + 1) * D],
                )

            def load_x_cols(lo, hi):
                for kc in range(KCH):
                    nc.sync.dma_start(
                        xsb[kc][:, lo:hi],
                        xh[:, kc * rows + lo : kc * rows + hi],
                    )

            # loads merged in first-use order: for each tile, the experts
            # it introduces and the x-column block covering it must have
            # been issued; emit W loads and 4-tile x blocks sorted by the
            # tile that first needs them.
            e0, e1 = pair_order[0]
            load_w_kc(e0, 0)

            # PE warmup: the HAM clock gate defaults to 1.2 GHz and only
            # releases after ~3.4us of sustained PE activity; the first PE
            # instruction also pays the cold IRAM fetch. Run dummy matmuls
            # against the first-loaded W chunk during the DMA window so
            # the real matmuls start warm.
            warm_ps = pspool.tile([128, D], f32, tag="warm", bufs=1)
            for _ in range(8):
                nc.tensor.matmul(
                    warm_ps[0:64, :],
                    w_tiles[e0][:, 0:64],
                    w_tiles[e0][:, 0:D],
                    start=True,
                    stop=True,
                )

            load_w_kc(e1, 0)
            for kc in range(KCH):
                nc.sync.dma_start(
                    xsb[kc][:, 0 : offs[1]],
                    xh[:, kc * rows : kc * rows + offs[1]],
                )
            for kc in range(1, KCH):
                load_w_kc(e0, kc)
                load_w_kc(e1, kc)
            nc.sync.dma_start(pw_all[:], pwa[:, :])

            events = []  # (first_use_tile, priority, kind, arg)
            seen = {e0, e1}
            for t, pr in enumerate(pair_order):
                for e in pr:
                    if e not in seen:
                        seen.add(e)
                        # lead the first use by 4 tiles: a matmul whose
                        # weight sem-wait is unresolved also loses the
                        # LDWEIGHTS pull-ahead (427ns instead of 234ns)
                        events.append((max(t - 4, 0), 0, "w", e))
            nblk = (G + 3) // 4
            for k in range(nblk):
                t0 = 4 * k
                if t0 == 0:
                    lo, hi = int(offs[1]), int(offs[4])
                else:
                    lo, hi = int(offs[t0]), int(offs[min(t0 + 4, G)])
                events.append((max(t0 - 3, 0), 1, "x", (lo, hi)))
            events.sort()
            for _, _, kind, arg in events:
                if kind == "w":
                    load_w(arg)
                else:
                    load_x_cols(*arg)

            for g, (a, b) in enumerate(pair_order):
                m = int(gcaps[g])
                lo, hi = int(offs[g]), int(offs[g] + m)
                pa = pspool.tile([TCAP, D], f32, tag="pa")
                pb = pspool.tile([TCAP, D], f32, tag="pb")
                for kc in range(KCH):
                    # both experts consume the same stationary x chunk
                    # back-to-back so walrus's ldw-opt can skip the reload
                    nc.tensor.matmul(
                        pa[0:m, :],
                        xsb[kc][:, lo:hi],
                        w_tiles[a][:, kc * D : (kc + 1) * D],
                        start=(kc == 0),
                        stop=(kc == KCH - 1),
                    )
                    nc.tensor.matmul(
                        pb[0:m, :],
                        xsb[kc][:, lo:hi],
                        w_tiles[b][:, kc * D : (kc + 1) * D],
                        start=(kc == 0),
                        stop=(kc == KCH - 1),
                    )
                # combine: out = pa*w_lo + pb*w_hi, split across ACT and DVE
                tmp = vpool.tile([TCAP, D], f32)
                nc.scalar.activation(
                    tmp[0:m, :],
                    pb[0:m, :],
                    mybir.ActivationFunctionType.Copy,
                    scale=pw_all[0:m, 2 * g + 1 : 2 * g + 2],
                )
                o = opool.tile([TCAP, D], f32)
                nc.vector.scalar_tensor_tensor(
                    o[0:m, :],
                    pa[0:m, :],
                    pw_all[0:m, 2 * g : 2 * g + 1],
                    tmp[0:m, :],
                    mybir.AluOpType.mult,
                    mybir.AluOpType.add,
                )
                nc.gpsimd.dma_start(y[lo:hi, :], o[0:m, :])
    _split_multi_waits(nc)
    return nc


def _assign(indices, probabilities, cap=TCAP):
    """Build per-core row assignments for every (token, gate) pair.

    Returns rows[c] = list of (token, group, w_lo, w_hi). Normal path:
    each token appears exactly once (both its gates land in the group of
    its expert pair). Groups whose per-core share exceeds TCAP, and
    duplicate-expert tokens, fall back to two single-gate rows placed in
    the least-loaded groups containing that expert.
    """
    gid = {p: g for g, p in enumerate(PAIRS)}
    groups_of = [[g for g, pr in enumerate(PAIRS) if e in pr] for e in range(E)]
    idx0, idx1 = indices[:, 0].astype(np.int64), indices[:, 1].astype(np.int64)
    p0, p1 = probabilities[:, 0], probabilities[:, 1]
    lo = np.minimum(idx0, idx1)
    hi = np.maximum(idx0, idx1)
    w_lo = np.where(idx0 <= idx1, p0, p1)
    w_hi = np.where(idx0 <= idx1, p1, p0)

    entries = [[] for _ in range(G)]  # group -> list of (token, w_lo, w_hi)
    singles = []  # (token, expert, weight) fallback entries
    dup = lo == hi
    for n in np.nonzero(dup)[0]:
        singles.append((int(n), int(lo[n]), float(p0[n] + p1[n])))
    ok = np.nonzero(~dup)[0]
    gids = np.array([gid[(int(a), int(b))] for a, b in zip(lo[ok], hi[ok])])
    for g in range(G):
        for n in ok[gids == g]:
            entries[g].append((int(n), float(w_lo[n]), float(w_hi[n])))

    rows = [[] for _ in range(NCORES)]  # core -> (token, group, wl, wh)
    used = np.zeros((NCORES, G), np.int64)
    for g in range(G):
        for j, (n, wl, wh) in enumerate(entries[g]):
            c = j % NCORES
            if used[c, g] < cap:
                rows[c].append((n, g, wl, wh))
                used[c, g] += 1
            else:
                a, b = PAIRS[g]
                singles.append((n, a, wl))
                singles.append((n, b, wh))
    for n, e, w in singles:
        # least-loaded (core, group) slot among groups containing e
        cands = [
            (used[c, g], c, g)
            for c in range(NCORES)
            for g in groups_of[e]
            if used[c, g] < cap
        ]
        assert cands, "no capacity left for fallback entry"
        _, c, g = min(cands)
        a, b = PAIRS[g]
        wl, wh = (w, 0.0) if e == a else (0.0, w)
        rows[c].append((n, g, wl, wh))
        used[c, g] += 1
    return rows, used


def kernel(input_batch, probabilities, indices, W, b, **_unused):
    global LAST_EXEC_TIME_NS
    x = np.ascontiguousarray(np.asarray(input_batch, dtype=np.float32))
    p = np.ascontiguousarray(np.asarray(probabilities, dtype=np.float32))
    idx = np.asarray(indices)
    Wf = np.ascontiguousarray(np.asarray(W, dtype=np.float32))
    bf = np.asarray(b, dtype=np.float32)
    assert x.shape == (N, D) and p.shape == (N, TOPK)
    assert idx.shape == (N, TOPK) and Wf.shape == (E, D, D)

    import os as _os
    _fix = int(_os.environ.get("MOE_FIXED_GCAP", "0"))
    rows, used = _assign(idx, p, cap=_fix if _fix else TCAP)
    # per-group M = max per-core occupancy quantized to {96, 128}: matmul
    # cost is free-dim-bound (M-independent), but issuing matmuls whose
    # stationary shape differs from the previous one costs ~47ns each, so
    # keep shapes uniform within two classes (single transition). Ranges
    # stay 32-aligned for DMA descriptor efficiency.
    occ = np.maximum(used.max(axis=0), 1)
    gcaps = np.where(occ <= 64, 64, np.where(occ <= 96, 96, TCAP)).astype(
        np.int64
    )
    if _fix:
        gcaps = np.full(G, _fix, np.int64)
    # order: 96-class tiles first in triangle order (experts stream in
    # gradually), then the 128-class (chain pairs) once all weights are
    # resident
    order = np.array(
        sorted(range(G), key=lambda g: (int(gcaps[g]), g)), dtype=np.int64
    )
    gcaps = gcaps[order]
    pair_order = [PAIRS[int(g)] for g in order]
    pos = np.empty(G, np.int64)  # original group id -> tile index
    pos[order] = np.arange(G)
    offs = np.concatenate([[0], np.cumsum(gcaps)]).astype(int)
    rows_total = int(offs[-1])

    # [p, (e, kc, dout)] layout; see _build_bass
    wdma = np.ascontiguousarray(
        Wf.reshape(E, KCH, 128, D).transpose(2, 0, 1, 3).reshape(128, E * KCH * D)
    )

    in_maps = []
    tok_maps = []
    for c in range(NCORES):
        x_rows = np.zeros((rows_total, D), np.float32)
        pw_arr = np.zeros((rows_total, 2), np.float32)
        tok_arr = np.full(rows_total, -1, np.int64)
        slot_used = np.zeros(G, np.int64)
        for n, g, wl, wh in rows[c]:
            t = int(pos[g])
            s = int(offs[t] + slot_used[t])
            slot_used[t] += 1
            x_rows[s] = x[n]
            pw_arr[s, 0] = wl
            pw_arr[s, 1] = wh
            tok_arr[s] = n
        # xh: [p, (kc, col)]; see _build_bass
        xh = (
            x_rows.reshape(rows_total, KCH, 128)
            .transpose(2, 1, 0)
            .reshape(128, KCH * rows_total)
        )
        pwa = np.zeros((TCAP, G * 2), np.float32)
        for t in range(G):
            m = int(gcaps[t])
            pwa[0:m, 2 * t : 2 * t + 2] = pw_arr[offs[t] : offs[t] + m]
        in_maps.append(
            {
                "xh": np.ascontiguousarray(xh),
                "pwa": pwa,
                "wdma": wdma,
            }
        )
        tok_maps.append(tok_arr)

    key = (tuple(int(v) for v in gcaps), tuple(pair_order))
    if _cache.get("key") != key:
        _cache["nc"] = _build_bass(gcaps, pair_order)
        _cache["key"] = key
    nc = _cache["nc"]

    res = run_bass_kernel_spmd(nc, in_maps, list(range(NCORES)))
    LAST_EXEC_TIME_NS = res.exec_time_ns

    out = np.zeros((N, D), np.float32)
    all_tok = np.concatenate(tok_maps)
    all_y = np.concatenate([res.results[c]["y"] for c in range(NCORES)], axis=0)
    valid = all_tok >= 0
    vt = all_tok[valid]
    counts = np.bincount(vt, minlength=N)
    if counts.max() <= 1:
        out[vt] = all_y[valid]
    else:
        np.add.at(out, vt, all_y[valid])

    if np.any(bf):
        # gate-weighted bias: out[n] += sum_k p[n,k] * b[idx[n,k]]
        mask = np.zeros((N, E), np.float32)
        np.add.at(mask, (np.arange(N)[:, None], idx.astype(np.int64)), p)
        out += mask @ bf

    total_loss = np.float32(0.0)
    return out, total_loss
